# revision 1
# baseline (speedup 1.0000x reference)
"""Trainium2 Bass kernel for nn_GatedAttention (linear attention with sigmoid
gate).

Strategy: shard the 16384 token rows across 8 cores (2048 each; cores 2b,2b+1
hold batch b). Per core, two phases:
  A: K,V projections (token-major) + per-head kv' = K^T [V|1] accumulated in
     PSUM over all local tokens (the ones column folds k_sum into kv').
  -- pairwise AllReduce of kv' between the two cores sharing a batch --
  B: Q,G projections (feature-major), out^T = kv'^T @ Q per head, normalizer
     z = SCALE/max(q.k_sum,eps) applied via tiny selector matmuls, gate, and
     the final output projection, all feature-major.
Host transposes x to feature-major and pre-transposes weights; output returns
feature-major per-core slabs that the host transposes back.
"""
import sys

sys.path.insert(0, "/opt/trn_rl_repo")

import numpy as np
import ml_dtypes

B, N, DIM = 4, 4096, 1024
HEADS, DH = 16, 64
SCALE = DH ** -0.5
N_CORES = 8
TPC = B * N // N_CORES      # 2048 tokens per core
NMT = TPC // 128            # 16 m-tiles (phase A)
CHUNK = 512
NCH = TPC // CHUNK          # 4 chunks (phase B)
CLAMP = 1e-6 / SCALE

DT_MODE = "bf16"            # "bf16" | "f32r" | "f32"

_CACHE = {}


def _build(dt_mode=DT_MODE, reps=1):
    import concourse.bacc as bacc
    import concourse.bass as bass
    import concourse.tile as tile
    from concourse import mybir

    AF = mybir.ActivationFunctionType
    F32 = mybir.dt.float32
    DT = mybir.dt.bfloat16 if dt_mode == "bf16" else mybir.dt.float32

    def mm(ap):
        # matmul-operand view: reduced-precision f32 mode uses float32r APs
        return ap.bitcast(mybir.dt.float32r) if dt_mode == "f32r" else ap

    ts = bass.ts

    nc = bacc.Bacc("TRN2", target_bir_lowering=False, debug=False,
                   num_devices=N_CORES)
    xt = nc.dram_tensor("xt", [DIM, TPC], DT, kind="ExternalInput")
    w_in = {}
    for nm in ("wk", "wv", "wq", "wg", "wo"):
        w_in[nm] = nc.dram_tensor(nm, [DIM, DIM], DT, kind="ExternalInput")
    bg_d = nc.dram_tensor("bg", [DIM], F32, kind="ExternalInput")
    y_d = nc.dram_tensor("y", [DIM, TPC], F32, kind="ExternalOutput")
    cc_in = nc.dram_tensor("cc_in", [128, 8, 65], F32)
    cc_out = nc.dram_tensor("cc_out", [128, 8, 65], F32)

    with tile.TileContext(nc, num_cores=N_CORES) as tc:
        with (
            tc.tile_pool(name="persist", bufs=1) as persist,
            tc.tile_pool(name="pb_big", bufs=2) as pb_big,
        ):
            X = persist.tile([128, 8, TPC], DT, tag="x")
            for i in range(8):
                nc.sync.dma_start(out=X[:, i, :], in_=xt.ap()[ts(i, 128), :])
            wsb = {}
            for nm in ("wq", "wg", "wo"):
                wsb[nm] = persist.tile([128, 8, DIM], DT, tag=nm, name=nm)
                for i in range(8):
                    nc.sync.dma_start(out=wsb[nm][:, i, :],
                                      in_=w_in[nm].ap()[ts(i, 128), :])
            bg_sb = persist.tile([128, 8], F32, tag="bg")
            bg_ap = bg_d.ap()
            nc.sync.dma_start(
                out=bg_sb[:],
                in_=bass.AP(tensor=bg_ap.tensor, offset=0,
                            ap=[[1, 128], [128, 8]]),
            )
            sel_np = np.zeros((16, 8, 128), _np_dt(dt_mode))
            for p in range(8):
                sel_np[2 * p, p, 0:64] = 1.0
                sel_np[2 * p + 1, p, 64:128] = 1.0
            sel_d = nc.inline_tensor(sel_np, name="sel_const")
            sel = persist.tile([16, 8, 128], DT, tag="sel")
            nc.sync.dma_start(out=sel[:], in_=sel_d.ap())

            for _rep in range(reps):
                _phases(nc, tc, bass, mybir, AF, F32, DT, mm, ts, X, wsb,
                        bg_sb, sel, w_in, cc_in, cc_out, y_d, tc_pools=(persist, pb_big))
    nc.compile()
    return nc


def _phases(nc, tc, bass, mybir, AF, F32, DT, mm, ts, X, wsb, bg_sb, sel,
            w_in, cc_in, cc_out, y_d, tc_pools):
    persist, pb_big = tc_pools
    if True:
        if True:
            # ---------------- phase A ----------------
            with (
                tc.tile_pool(name="pa_w", bufs=1) as pa_w,
                tc.tile_pool(name="pa_tmp", bufs=2) as pa_tmp,
                tc.tile_pool(name="pa_ps", bufs=2, space="PSUM") as pa_ps,
                tc.tile_pool(name="kv_ps", bufs=1, space="PSUM") as kv_pool,
            ):
                for nm in ("wk", "wv"):
                    wsb[nm] = pa_w.tile([128, 8, DIM], DT, tag=nm, name=nm)
                    for i in range(8):
                        nc.sync.dma_start(out=wsb[nm][:, i, :],
                                          in_=w_in[nm].ap()[ts(i, 128), :])
                kv_acc = pa_tmp.tile([128, 8, 65], F32, tag="kv_acc",
                                     bufs=1, name="kv_acc")
                nc.vector.memset(kv_acc[:], 0.0)
                for mt in range(NMT):
                    msl = ts(mt, 128)
                    kps = pa_ps.tile([128, 1024], F32, tag="proj")
                    for i in range(8):
                        for o in range(2):
                            nc.tensor.matmul(
                                kps[:, ts(o, 512)],
                                mm(X[:, i, msl]),
                                mm(wsb["wk"][:, i, ts(o, 512)]),
                                start=(i == 0), stop=(i == 7),
                            )
                    r1 = pa_tmp.tile([128, 1024], F32, tag="r1")
                    nc.scalar.activation(r1, kps, AF.Relu)
                    m1 = pa_tmp.tile([128, 1024], F32, tag="m1")
                    nc.vector.tensor_scalar_min(m1, kps, 0.0)
                    e1 = pa_tmp.tile([128, 1024], F32, tag="e1")
                    nc.scalar.activation(e1, m1, AF.Exp)
                    ksb = pa_tmp.tile([128, 1024], DT, tag="ksb")
                    nc.vector.tensor_add(ksb, r1, e1)

                    vps = pa_ps.tile([128, 16, 64], F32, tag="proj")
                    for i in range(8):
                        for o in range(2):
                            nc.tensor.matmul(
                                vps[:, ts(o, 8), :],
                                mm(X[:, i, msl]),
                                mm(wsb["wv"][:, i, ts(o, 512)]),
                                start=(i == 0), stop=(i == 7),
                            )
                    vp = pa_tmp.tile([128, 16, 65], DT, tag="vp")
                    nc.vector.memset(vp[:, :, 64:65], 1.0)
                    nc.scalar.copy(vp[:, :, 0:64], vps[:, :, :])

                    for w in range(2):
                        kvws = []
                        for j in range(4):
                            kvw = kv_pool.tile([128, 65], F32, tag="kvw",
                                               bufs=4, name="kvw")
                            kvws.append(kvw)
                            for c in range(2):
                                h = 8 * w + 2 * j + c
                                nc.tensor.matmul(
                                    kvw[64 * c:64 * c + 64, :],
                                    mm(ksb[:, ts(h, 64)]),
                                    mm(vp[:, h, :]),
                                    start=True, stop=True,
                                )
                        for j in range(4):
                            nc.vector.tensor_add(
                                kv_acc[:, 4 * w + j, :],
                                kv_acc[:, 4 * w + j, :], kvws[j][:])
                nc.sync.dma_start(out=cc_in.ap()[:, :, :], in_=kv_acc[:])

            nc.gpsimd.collective_compute(
                "AllReduce",
                mybir.AluOpType.add,
                replica_groups=[[0, 1], [2, 3], [4, 5], [6, 7]],
                ins=[cc_in.ap().opt()],
                outs=[cc_out.ap().opt()],
            )

            # ---------------- phase B ----------------
            with (
                tc.tile_pool(name="pb_tmp", bufs=2) as pb_tmp,
                tc.tile_pool(name="pb_small", bufs=1) as pb_small,
                tc.tile_pool(name="ps_proj", bufs=2, space="PSUM") as ps_proj,
                tc.tile_pool(name="ps_misc", bufs=4, space="PSUM") as ps_misc,
                tc.tile_pool(name="ps_y", bufs=2, space="PSUM") as ps_y,
            ):
                kvf = pb_small.tile([128, 8, 65], F32, tag="kvf")
                nc.sync.dma_start(out=kvf[:], in_=cc_out.ap()[:, :, :])
                kvb = pb_small.tile([128, 8, 65], DT, tag="kvb")
                nc.vector.tensor_copy(kvb, kvf)
                ksd = pb_small.tile([128, 8, 16], DT, tag="ksd")
                nc.vector.memset(ksd[:], 0.0)
                for p in range(8):
                    nc.scalar.activation(ksd[0:64, p, 2 * p:2 * p + 1],
                                         kvf[0:64, p, 64:65],
                                         AF.Copy, scale=1.0 / SCALE)
                    nc.scalar.activation(ksd[64:128, p, 2 * p + 1:2 * p + 2],
                                         kvf[64:128, p, 64:65],
                                         AF.Copy, scale=1.0 / SCALE)

                for ch in range(NCH):
                    csl = ts(ch, CHUNK)
                    qsb = pb_big.tile([128, 8, CHUNK], DT, tag="qsb")
                    gsb = pb_big.tile([128, 8, CHUNK], DT, tag="gsb")
                    for p in range(8):
                        qps = ps_proj.tile([128, CHUNK], F32, tag="proj")
                        for i in range(8):
                            nc.tensor.matmul(
                                qps, mm(wsb["wq"][:, i, ts(p, 128)]),
                                mm(X[:, i, csl]),
                                start=(i == 0), stop=(i == 7),
                            )
                        r1 = pb_tmp.tile([128, CHUNK], F32, tag="br1")
                        nc.scalar.activation(r1, qps, AF.Relu)
                        m1 = pb_tmp.tile([128, CHUNK], F32, tag="bm1")
                        nc.vector.tensor_scalar_min(m1, qps, 0.0)
                        e1 = pb_tmp.tile([128, CHUNK], F32, tag="be1")
                        nc.scalar.activation(e1, m1, AF.Exp)
                        nc.vector.tensor_add(qsb[:, p, :], r1, e1)

                        gps = ps_proj.tile([128, CHUNK], F32, tag="proj")
                        for i in range(8):
                            nc.tensor.matmul(
                                gps, mm(wsb["wg"][:, i, ts(p, 128)]),
                                mm(X[:, i, csl]),
                                start=(i == 0), stop=(i == 7),
                            )
                        nc.scalar.activation(gsb[:, p, :], gps, AF.Sigmoid,
                                             bias=bg_sb[:, p:p + 1])

                    qkps = ps_misc.tile([16, CHUNK], F32, tag="misc")
                    for p in range(8):
                        nc.tensor.matmul(
                            qkps, mm(ksd[:, p, :]), mm(qsb[:, p, :]),
                            start=(p == 0), stop=(p == 7),
                            skip_group_check=True,
                        )
                    zq = pb_tmp.tile([16, CHUNK], F32, tag="zq")
                    nc.vector.tensor_scalar_max(zq, qkps, CLAMP)
                    zr = pb_tmp.tile([16, CHUNK], F32, tag="zr")
                    nc.vector.reciprocal(zr, zq)
                    zqr = pb_tmp.tile([16, CHUNK], DT, tag="zqr")
                    nc.vector.tensor_copy(zqr, zr)

                    asb = pb_big.tile([128, 8, CHUNK], DT, tag="asb")
                    for p in range(8):
                        zbps = ps_misc.tile([128, CHUNK], F32, tag="misc")
                        nc.tensor.matmul(zbps, mm(sel[:, p, :]), mm(zqr),
                                         start=True, stop=True)
                        ops_ = ps_misc.tile([128, CHUNK], F32, tag="misc")
                        for rr in range(2):
                            pr = slice(64 * rr, 64 * rr + 64)
                            nc.tensor.matmul(
                                ops_[pr, :], mm(kvb[pr, p, 0:64]),
                                mm(qsb[pr, p, :]),
                                start=True, stop=True,
                            )
                        t1 = pb_tmp.tile([128, CHUNK], F32, tag="bt1")
                        nc.vector.tensor_mul(t1, ops_, gsb[:, p, :])
                        nc.vector.tensor_mul(asb[:, p, :], t1, zbps)

                    for d in range(8):
                        yps = ps_y.tile([128, CHUNK], F32, tag="y")
                        for fi in range(8):
                            nc.tensor.matmul(
                                yps, mm(wsb["wo"][:, fi, ts(d, 128)]),
                                mm(asb[:, fi, :]),
                                start=(fi == 0), stop=(fi == 7),
                            )
                        ysb = pb_tmp.tile([128, CHUNK], F32, tag="ysb")
                        nc.scalar.copy(ysb, yps)
                        nc.sync.dma_start(out=y_d.ap()[ts(d, 128), csl],
                                          in_=ysb[:])


def _np_dt(dt_mode):
    return ml_dtypes.bfloat16 if dt_mode == "bf16" else np.float32


def prep_inputs(x, Wq, Wk, Wv, Wg, bg, Wo, dt_mode=DT_MODE):
    npdt = _np_dt(dt_mode)
    x_f = np.ascontiguousarray(np.asarray(x, np.float32).reshape(B * N, DIM))
    w_t = {}
    for nm, W in (("wq", Wq), ("wk", Wk), ("wv", Wv), ("wg", Wg)):
        w_t[nm] = np.ascontiguousarray(
            np.asarray(W, np.float32).T).astype(npdt)
    w_t["wo"] = np.ascontiguousarray(
        np.asarray(Wo, np.float32).T).astype(npdt)
    bg_f = np.ascontiguousarray(np.asarray(bg, np.float32))
    in_maps = []
    for c in range(N_CORES):
        xt_c = np.ascontiguousarray(
            x_f[c * TPC:(c + 1) * TPC].T).astype(npdt)
        m = {"xt": xt_c, "bg": bg_f}
        m.update(w_t)
        in_maps.append(m)
    return in_maps


def unshard_output(y_parts):
    out = np.empty((B * N, DIM), np.float32)
    for c in range(N_CORES):
        out[c * TPC:(c + 1) * TPC] = np.asarray(y_parts[c]).T
    return out.reshape(B, N, DIM)


def get_nc(dt_mode=DT_MODE):
    key = ("nc", dt_mode)
    if key not in _CACHE:
        _CACHE[key] = _build(dt_mode)
    return _CACHE[key]


def kernel(x, Wq, Wk, Wv, Wg, bg, Wo):
    from concourse.bass_utils import run_bass_kernel_spmd

    nc = get_nc()
    in_maps = prep_inputs(x, Wq, Wk, Wv, Wg, bg, Wo)
    res = run_bass_kernel_spmd(nc, in_maps, core_ids=list(range(N_CORES)))
    return unshard_output([res.results[c]["y"] for c in range(N_CORES)])



# revision 20
# speedup vs baseline: 48.6905x; 48.6905x over previous
"""Trainium2 Bass kernel for nn_GatedAttention (linear attention with sigmoid
gate).

Strategy: shard the 16384 token rows across 8 cores (2048 each; cores 2b,2b+1
hold batch b). Per core:
  A: K,V projections (token-major) + per-head kv' = K^T [V|1] accumulated over
     all local tokens (the ones column folds k_sum into kv').
  -- pairwise AllReduce of kv' between the two cores sharing a batch,
     overlapped with the start of phase B --
  B: per 512-token chunk: Q,G projections (feature-major), then (lagging
     LOOKAHEAD chunks so the collective hides) out^T = kv'^T @ Q per head,
     normalizer z = SCALE/max(q.k_sum,eps) via tiny selector matmuls, gate,
     and the final output projection.
Engine balance: scalar runs only Exp/Sigmoid (batched per chunk to avoid
activation-table reloads); relu/min/add run on DVE; psum->sbuf copies and the
kv accumulation adds run on gpsimd.
Host transposes x to feature-major and pre-transposes weights; output returns
feature-major per-core slabs that the host transposes back.
"""
import sys

sys.path.insert(0, "/opt/trn_rl_repo")

import numpy as np
import ml_dtypes

B, N, DIM = 4, 4096, 1024
HEADS, DH = 16, 64
SCALE = DH ** -0.5
N_CORES = 8
TPC = B * N // N_CORES      # 2048 tokens per core
NMT = TPC // 128            # 16 m-tiles (phase A)
CHUNK = 512
NCH = TPC // CHUNK          # 4 chunks (phase B)
CLAMP = 1e-6 / SCALE
LOOKAHEAD = 2               # chunks of Q/G proj emitted before attn starts (qsb/gsb bufs must be LOOKAHEAD+1)

DT_MODE = "bf16"            # "bf16" | "f32r" | "f32"

_CACHE = {}


def _build(dt_mode=DT_MODE, reps=1):
    import concourse.bacc as bacc
    import concourse.bass as bass
    import concourse.tile as tile
    from concourse import mybir

    AF = mybir.ActivationFunctionType
    F32 = mybir.dt.float32
    DT = mybir.dt.bfloat16 if dt_mode == "bf16" else mybir.dt.float32

    def mm(ap):
        return ap.bitcast(mybir.dt.float32r) if dt_mode == "f32r" else ap

    ts = bass.ts

    nc = bacc.Bacc("TRN2", target_bir_lowering=False, debug=False,
                   num_devices=N_CORES)
    xt = nc.dram_tensor("xt", [DIM, TPC], DT, kind="ExternalInput")
    w_in = {}
    for nm in ("wk", "wv", "wq", "wg", "wo"):
        w_in[nm] = nc.dram_tensor(nm, [DIM, DIM], DT, kind="ExternalInput")
    bg_d = nc.dram_tensor("bg", [DIM], F32, kind="ExternalInput")
    y_d = nc.dram_tensor("y", [DIM, TPC], DT, kind="ExternalOutput")
    cc_in = nc.dram_tensor("cc_in", [128, 8, 65], F32)
    cc_out = nc.dram_tensor("cc_out", [128, 8, 65], F32)

    with tile.TileContext(nc, num_cores=N_CORES) as tc:
        with (
            tc.tile_pool(name="persist", bufs=1) as persist,
            tc.tile_pool(name="qg", bufs=1) as qgpool,
            tc.tile_pool(name="atmp", bufs=2) as atmp,
            tc.tile_pool(name="btmp", bufs=2) as btmp,
        ):
            wsb = {}
            for nm in ("wk", "wv", "wq", "wg", "wo"):
                wsb[nm] = persist.tile([128, 8, DIM], DT, tag=nm, name=nm)
            X = persist.tile([128, 8, TPC], DT, tag="x")

            def _ldw(nm, o):
                for i in range(8):
                    nc.sync.dma_start(out=wsb[nm][:, i, ts(o, 512)],
                                      in_=w_in[nm].ap()[ts(i, 128), ts(o, 512)])

            def _ldx(e):
                for i in range(8):
                    nc.sync.dma_start(out=X[:, i, ts(e, 256)],
                                      in_=xt.ap()[ts(i, 128), ts(e, 256)])

            # gate set for the first matmuls lands first
            _ldw("wk", 0)
            _ldx(0)
            _ldw("wk", 1)
            _ldw("wv", 0)
            _ldw("wv", 1)
            for e in range(1, 8):
                _ldx(e)
            for nm in ("wq", "wg", "wo"):
                for o in range(2):
                    _ldw(nm, o)
            bg_sb = persist.tile([128, 8], F32, tag="bg")
            bg_ap = bg_d.ap()
            nc.sync.dma_start(
                out=bg_sb[:],
                in_=bass.AP(tensor=bg_ap.tensor, offset=0,
                            ap=[[1, 128], [128, 8]]),
            )
            bgh = persist.tile([128, 8], F32, tag="bgh", name="bgh")
            nc.scalar.activation(bgh, bg_sb, AF.Copy, scale=0.5)
            sel_np = np.zeros((16, 8, 128), _np_dt(dt_mode))
            for p in range(8):
                sel_np[2 * p, p, 0:64] = 1.0
                sel_np[2 * p + 1, p, 64:128] = 1.0
            sel_d = nc.inline_tensor(sel_np, name="sel_const")
            sel = persist.tile([16, 8, 128], DT, tag="sel")
            nc.sync.dma_start(out=sel[:], in_=sel_d.ap())

            for _rep in range(reps):
                _phases(nc, tc, bass, mybir, AF, F32, DT, mm, ts, X, wsb,
                        bgh, sel, cc_in, cc_out, y_d,
                        pools=(persist, qgpool, atmp, btmp))
    nc.compile()
    return nc


def _phases(nc, tc, bass, mybir, AF, F32, DT, mm, ts, X, wsb, bgh, sel,
            cc_in, cc_out, y_d, pools):
    persist, qgpool, atmp, btmp = pools

    # ---------------- phase A: K,V proj + kv accumulation ----------------
    with (
        tc.tile_pool(name="pa_ps", bufs=2, space="PSUM") as pa_ps,
        tc.tile_pool(name="kv_ps", bufs=1, space="PSUM") as kv_pool,
    ):
        kv_acc = atmp.tile([128, 8, 65], F32, tag="kv_acc", bufs=1,
                           name="kv_acc")
        nc.vector.memset(kv_acc[:], 0.0)
        for mt in range(NMT):
            msl = ts(mt, 128)
            kps = pa_ps.tile([128, 1024], F32, tag="proj")
            for o in range(2):
                for i in range(8):
                    nc.tensor.matmul(
                        kps[:, ts(o, 512)],
                        mm(X[:, i, msl]),
                        mm(wsb["wk"][:, i, ts(o, 512)]),
                        start=(i == 0), stop=(i == 7),
                    )
            # elu(k)+1 = max(k,0) + exp(min(k,0)); scalar engine does only Exp
            r1 = atmp.tile([128, 1024], DT, tag="r1")
            nc.scalar.activation(r1, kps, AF.Relu)
            m1 = atmp.tile([128, 1024], DT, tag="m1")
            nc.vector.tensor_scalar_min(m1, kps, 0.0)
            nc.scalar.activation(m1, m1, AF.Exp)
            ksb = atmp.tile([128, 1024], DT, tag="ksb")
            nc.vector.tensor_add(ksb, r1, m1)

            vps = pa_ps.tile([128, 16, 64], F32, tag="proj")
            for i in range(8):
                for o in range(2):
                    nc.tensor.matmul(
                        vps[:, ts(o, 8), :],
                        mm(X[:, i, msl]),
                        mm(wsb["wv"][:, i, ts(o, 512)]),
                        start=(i == 0), stop=(i == 7),
                    )
            vp = atmp.tile([128, 16, 65], DT, tag="vp")
            nc.vector.memset(vp[:, :, 64:65], 1.0)
            nc.scalar.copy(vp[:, :, 0:64], vps[:, :, :])

            for w in range(2):
                kvws = []
                for j in range(4):
                    kvw = kv_pool.tile([128, 65], F32, tag="kvw",
                                       bufs=4, name="kvw")
                    kvws.append(kvw)
                    for c in range(2):
                        h = 8 * w + 2 * j + c
                        nc.tensor.matmul(
                            kvw[64 * c:64 * c + 64, :],
                            mm(ksb[:, ts(h, 64)]),
                            mm(vp[:, h, :]),
                            start=True, stop=True,
                        )
                for j in range(4):
                    nc.vector.tensor_add(
                        kv_acc[:, 4 * w + j, :],
                        kv_acc[:, 4 * w + j, :], kvws[j][:])

    # ---------------- collective (async, hidden under phase B) ----------
    with (
        tc.tile_pool(name="ps_proj", bufs=2, space="PSUM") as ps_proj,
        tc.tile_pool(name="ps_misc", bufs=4, space="PSUM") as ps_misc,
        tc.tile_pool(name="ps_y", bufs=2, space="PSUM") as ps_y,
    ):
        nc.sync.dma_start(out=cc_in.ap()[:, :, :], in_=kv_acc[:])
        nc.gpsimd.collective_compute(
            "AllReduce",
            mybir.AluOpType.add,
            replica_groups=[[0, 1], [2, 3], [4, 5], [6, 7]],
            ins=[cc_in.ap().opt()],
            outs=[cc_out.ap().opt()],
        )

        kvstate = {}

        def kv_fetch(qsb_gate):
            # g0 is always zero but *depends on* the last lookahead chunk's
            # qsb, so every CC-blocked op below is scheduled after the
            # lookahead QG work on its engine queue (no head-of-line block
            # while the collective is still in flight).
            g0 = btmp.tile([128, 1], F32, tag="g0", bufs=1, name="g0")
            nc.vector.tensor_scalar_mul(g0, qsb_gate[:, 7, 0:1], 0.0)
            kvf = btmp.tile([128, 8, 65], F32, tag="kvf", bufs=1, name="kvf")
            nc.sync.dma_start(out=kvf[:], in_=cc_out.ap()[:, :, :])
            kvb = btmp.tile([128, 8, 65], DT, tag="kvb", bufs=1, name="kvb")
            # kvb = kvf + g0 (g0 == 0, per-partition broadcast): the gate dep
            # keeps this CC-blocked op behind the lookahead QG drains on DVE
            nc.vector.tensor_scalar_add(kvb, kvf, g0[:, 0:1])
            g1 = btmp.tile([128, 1], F32, tag="g1", bufs=1, name="g1")
            nc.vector.tensor_scalar_add(g1, g0, 1.0 / SCALE)
            ksd = btmp.tile([128, 8, 16], DT, tag="ksd", bufs=1, name="ksd")
            nc.vector.memset(ksd[:], 0.0)
            for p in range(8):
                nc.scalar.activation(ksd[0:64, p, 2 * p:2 * p + 1],
                                     kvf[0:64, p, 64:65],
                                     AF.Copy, scale=g1[0:64, 0:1])
                nc.scalar.activation(ksd[64:128, p, 2 * p + 1:2 * p + 2],
                                     kvf[64:128, p, 64:65],
                                     AF.Copy, scale=g1[64:128, 0:1])
            kvstate["kvb"] = kvb
            kvstate["ksd"] = ksd

        # ---------------- phase B ----------------
        def qg_proj(ch):
            csl = ts(ch, CHUNK)
            qsb = qgpool.tile([128, 8, CHUNK], DT, tag="qsb", bufs=3,
                              name="qsb")
            gsb = qgpool.tile([128, 8, CHUNK], DT, tag="gsb", bufs=3,
                              name="gsb")
            for p in range(8):
                qps = ps_proj.tile([128, CHUNK], F32, tag="proj")
                for i in range(8):
                    nc.tensor.matmul(
                        qps, mm(wsb["wq"][:, i, ts(p, 128)]),
                        mm(X[:, i, csl]),
                        start=(i == 0), stop=(i == 7),
                    )
                r1 = btmp.tile([128, CHUNK], DT, tag="br1")
                nc.scalar.activation(r1, qps, AF.Relu)
                m1 = btmp.tile([128, CHUNK], DT, tag="bm1")
                nc.vector.tensor_scalar_min(m1, qps, 0.0)
                e1 = btmp.tile([128, CHUNK], DT, tag="be1")
                nc.scalar.activation(e1, m1, AF.Exp)
                nc.vector.tensor_add(qsb[:, p, :], r1, e1)
            for p in range(8):
                gps = ps_proj.tile([128, CHUNK], F32, tag="proj")
                for i in range(8):
                    nc.tensor.matmul(
                        gps, mm(wsb["wg"][:, i, ts(p, 128)]),
                        mm(X[:, i, csl]),
                        start=(i == 0), stop=(i == 7),
                    )
                gt = btmp.tile([128, CHUNK], DT, tag="bgt", bufs=1, name="bgt")
                nc.scalar.activation(gt, gps, AF.Tanh,
                                     bias=bgh[:, p:p + 1], scale=0.5)
                nc.vector.tensor_scalar(
                    gsb[:, p, :], gt, 0.5, 0.5,
                    mybir.AluOpType.mult, mybir.AluOpType.add)
            return qsb, gsb

        def attn(ch, qsb, gsb, qsb_gate):
            if "kvb" not in kvstate:
                kv_fetch(qsb_gate)
            kvb = kvstate["kvb"]
            ksd = kvstate["ksd"]
            csl = ts(ch, CHUNK)
            qkps = ps_misc.tile([16, CHUNK], F32, tag="misc")
            for p in range(8):
                nc.tensor.matmul(
                    qkps, mm(ksd[:, p, :]), mm(qsb[:, p, :]),
                    start=(p == 0), stop=(p == 7),
                    skip_group_check=True,
                )
            zq = btmp.tile([16, CHUNK], F32, tag="zq", bufs=1, name="zq")
            nc.vector.tensor_scalar_max(zq, qkps, CLAMP)
            zr = btmp.tile([16, CHUNK], F32, tag="zr", bufs=1, name="zr")
            nc.vector.reciprocal(zr, zq)
            zqr = btmp.tile([16, CHUNK], DT, tag="zqr", bufs=1, name="zqr")
            nc.vector.tensor_copy(zqr, zr)

            asb = btmp.tile([128, 8, CHUNK], DT, tag="asb", bufs=1, name="asb")
            for p in range(8):
                zbps = ps_misc.tile([128, CHUNK], F32, tag="misc")
                nc.tensor.matmul(zbps, mm(sel[:, p, :]), mm(zqr),
                                 start=True, stop=True)
                ops_ = ps_misc.tile([128, CHUNK], F32, tag="misc")
                for rr in range(2):
                    pr = slice(64 * rr, 64 * rr + 64)
                    nc.tensor.matmul(
                        ops_[pr, :], mm(kvb[pr, p, 0:64]),
                        mm(qsb[pr, p, :]),
                        start=True, stop=True,
                    )
                t1 = btmp.tile([128, CHUNK], DT, tag="bt1")
                nc.vector.tensor_mul(t1, ops_, gsb[:, p, :])
                nc.vector.tensor_mul(asb[:, p, :], t1, zbps)

            for d in range(8):
                yps = ps_y.tile([128, CHUNK], F32, tag="y")
                for fi in range(8):
                    nc.tensor.matmul(
                        yps, mm(wsb["wo"][:, fi, ts(d, 128)]),
                        mm(asb[:, fi, :]),
                        start=(fi == 0), stop=(fi == 7),
                    )
                ysb = btmp.tile([128, CHUNK], DT, tag="ysb")
                if d % 2 == 0:
                    nc.vector.tensor_copy(ysb, yps)
                else:
                    nc.scalar.copy(ysb, yps)
                nc.sync.dma_start(out=y_d.ap()[ts(d, 128), csl],
                                  in_=ysb[:])

        pend = []
        for ch in range(NCH):
            pend.append((ch, *qg_proj(ch)))
            if len(pend) >= LOOKAHEAD:
                attn(*pend.pop(0), qsb_gate=pend[-1][2])
        while pend:
            attn(*pend.pop(0), qsb_gate=pend[-1][2] if pend else None)


def _np_dt(dt_mode):
    return ml_dtypes.bfloat16 if dt_mode == "bf16" else np.float32


def prep_inputs(x, Wq, Wk, Wv, Wg, bg, Wo, dt_mode=DT_MODE):
    npdt = _np_dt(dt_mode)
    x_f = np.ascontiguousarray(np.asarray(x, np.float32).reshape(B * N, DIM))
    w_t = {}
    for nm, W in (("wq", Wq), ("wk", Wk), ("wv", Wv), ("wg", Wg)):
        w_t[nm] = np.ascontiguousarray(
            np.asarray(W, np.float32).T).astype(npdt)
    w_t["wo"] = np.ascontiguousarray(
        np.asarray(Wo, np.float32).T).astype(npdt)
    bg_f = np.ascontiguousarray(np.asarray(bg, np.float32))
    in_maps = []
    for c in range(N_CORES):
        xt_c = np.ascontiguousarray(
            x_f[c * TPC:(c + 1) * TPC].T).astype(npdt)
        m = {"xt": xt_c, "bg": bg_f}
        m.update(w_t)
        in_maps.append(m)
    return in_maps


def unshard_output(y_parts):
    out = np.empty((B * N, DIM), np.float32)
    for c in range(N_CORES):
        out[c * TPC:(c + 1) * TPC] = np.asarray(y_parts[c]).T
    return out.reshape(B, N, DIM)


def get_nc(dt_mode=DT_MODE):
    key = ("nc", dt_mode)
    if key not in _CACHE:
        _CACHE[key] = _build(dt_mode)
    return _CACHE[key]


def kernel(x, Wq, Wk, Wv, Wg, bg, Wo):
    from concourse.bass_utils import run_bass_kernel_spmd

    nc = get_nc()
    in_maps = prep_inputs(x, Wq, Wk, Wv, Wg, bg, Wo)
    res = run_bass_kernel_spmd(nc, in_maps, core_ids=list(range(N_CORES)))
    return unshard_output([res.results[c]["y"] for c in range(N_CORES)])
